# revision 7
# baseline (speedup 1.0000x reference)
"""BERT-LSTM-CRF kernel for Trainium2, 8 NeuronCores.

Wall-clock-optimized: under the axon tunnel the end-to-end time of a
warm kernel() call is dominated by host->device input bytes and
per-call compile/lowering overhead, not device compute. Design:

  * Batch-shard 8-way (8 samples/core); each core runs BOTH LSTM
    directions, so the aligned embeddings ship once (not once per
    direction-core as in the 4+4 direction split).
  * Ragged cut: the word aligner zero-pads past sent_len (<= 258 for
    this generator), so embeds rows t >= CUT(=272) are all-zero and
    xg degenerates to the bias row. Only [CUT*8, 768] embedding rows
    ship per core; the scans reuse the xg slice at t=CUT-1 (pure
    bias) for every step past the cut. Falls back to a full-length
    program if masks ever exceed the cut.
  * bf16 wire format for embeddings + weights (matmuls in bf16 with
    fp32 PSUM accumulation; cell state stays fp32).
  * Replicated weights (W_ih/W_hh/W_lin/bias, both directions) are
    sharded 1/8th per core on the host and AllGathered on device over
    NeuronLink, cutting their upload 8x.
  * fwd+bwd LSTM + output projection fused on device; output is the
    per-core feats [22, S*8] in bf16; host adds b_lin.
  * jax persistent compilation cache + memoized BIR serialization so
    warm calls skip the per-call NEFF recompile that otherwise costs
    seconds inside run_bass_kernel_spmd's fresh-jit path.

Per-call upload ~36MB vs ~305MB for the direction-split fp32 version.
"""
import os
import sys
import tempfile
import numpy as np

sys.path.insert(0, "/opt/trn_rl_repo")

B, S, D, H, T = 64, 512, 768, 384, 22
G4 = 4 * H            # 1536 gate rows
BL = 8                # batch per core
NC = 8
KD = D // 128         # 6 contraction chunks of the input GEMM
KH = H // 128         # 3 hidden chunks
CUT = 272             # compile-time ragged cut (>= max sent_len+1, x16)
STEPS = int(os.environ.get("KSTEPS", str(S)))
XCH = 4               # scan timesteps per xg DMA chunk

_cache = {}
_cfg_done = [False]

# gate-order permutation: torch [i,f,g,o] -> kernel [i,f,o,g]
_PERM = np.concatenate([np.arange(0, H), np.arange(H, 2 * H),
                        np.arange(3 * H, 4 * H), np.arange(2 * H, 3 * H)])


def _configure_jax_cache():
    """Persistent XLA compilation cache: the runner rebuilds its jit
    closure every call, so without this every warm call re-runs the
    multi-second NEFF compile."""
    if _cfg_done[0]:
        return
    _cfg_done[0] = True
    try:
        import jax
        jax.config.update(
            "jax_compilation_cache_dir",
            os.path.join(tempfile.gettempdir(), "jax_comp_cache"))
        jax.config.update("jax_persistent_cache_min_entry_size_bytes", -1)
        jax.config.update("jax_persistent_cache_min_compile_time_secs", 0.0)
    except Exception:
        pass


def _build_program(cut, steps, s_len=S):
    from concourse import bacc, tile, mybir
    from contextlib import ExitStack

    f32 = mybir.dt.float32
    bf16 = mybir.dt.bfloat16
    AF = mybir.ActivationFunctionType

    nc = bacc.Bacc("TRN2", target_bir_lowering=False, debug=False,
                   num_devices=NC)

    NR = (BL * cut) // 128   # emb row tiles

    emb = nc.dram_tensor("emb", [BL * cut, D], bf16, kind="ExternalInput")
    h0t = nc.dram_tensor("h0t", [2 * H, BL], bf16, kind="ExternalInput")
    c0 = nc.dram_tensor("c0", [BL, 2, H], f32, kind="ExternalInput")
    id128 = nc.dram_tensor("id128", [128, 128], bf16, kind="ExternalInput")
    id8 = nc.dram_tensor("id8", [8, 8], f32, kind="ExternalInput")
    # weight shards (1/8th each); full tensors assembled via AllGather
    wih_sh = nc.dram_tensor("wih_sh", [2 * D // NC, G4], bf16,
                            kind="ExternalInput")
    whh_sh = nc.dram_tensor("whh_sh", [2 * H // NC, G4], bf16,
                            kind="ExternalInput")
    wlin_sh = nc.dram_tensor("wlin_sh", [2 * H // NC, T], bf16,
                             kind="ExternalInput")
    bias_sh = nc.dram_tensor("bias_sh", [2 * G4 // NC], bf16,
                             kind="ExternalInput")
    wih_full = nc.dram_tensor("wih_full", [2 * D, G4], bf16,
                              addr_space="Shared")
    whh_full = nc.dram_tensor("whh_full", [2 * H, G4], bf16,
                              addr_space="Shared")
    wlin_full = nc.dram_tensor("wlin_full", [2 * H, T], bf16,
                               addr_space="Shared")
    bias_full = nc.dram_tensor("bias_full", [2, G4], bf16,
                               addr_space="Shared")
    out = nc.dram_tensor("out", [T, s_len * BL], bf16, kind="ExternalOutput")
    xg_dram = nc.dram_tensor("xg_scratch", [2, cut * BL, G4], f32)
    # collectives cannot read IO tensors: bounce shards through internal dram
    wih_bn = nc.dram_tensor("wih_bn", [2 * D // NC, G4], bf16)
    whh_bn = nc.dram_tensor("whh_bn", [2 * H // NC, G4], bf16)
    wlin_bn = nc.dram_tensor("wlin_bn", [2 * H // NC, T], bf16)
    bias_bn = nc.dram_tensor("bias_bn", [2 * G4 // NC], bf16)

    grp = [list(range(NC))]

    with tile.TileContext(nc) as tc, ExitStack() as big:
        for src, bn, full in ((wih_sh, wih_bn, wih_full),
                              (whh_sh, whh_bn, whh_full),
                              (wlin_sh, wlin_bn, wlin_full),
                              (bias_sh, bias_bn, bias_full)):
            nc.sync.dma_start(bn[:], src[:])
            nc.gpsimd.collective_compute(
                "AllGather", mybir.AluOpType.bypass, replica_groups=grp,
                ins=[bn[:]], outs=[full[:]])

        consts = big.enter_context(tc.tile_pool(name="consts", bufs=1))
        hist_pool = big.enter_context(tc.tile_pool(name="hist", bufs=1))

        id128_sb = consts.tile([128, 128], bf16, tag="id128")
        nc.sync.dma_start(id128_sb[:], id128[:])
        id8_sb = consts.tile([8, 8], f32, tag="id8")
        nc.sync.dma_start(id8_sb[:], id8[:])
        ones_sb = consts.tile([1, 128], bf16, tag="ones")
        nc.vector.memset(ones_sb[:], 1.0)

        whh_all = consts.tile([128, 2 * KH, G4], bf16, tag="whh")
        nc.sync.dma_start(whh_all[:],
                          whh_full.rearrange("(d k p) g -> p (d k) g",
                                             p=128, k=KH))
        wlin_all = consts.tile([128, 2 * KH, T], bf16, tag="wlin")
        nc.sync.dma_start(wlin_all[:],
                          wlin_full.rearrange("(d k p) t -> p (d k) t",
                                              p=128, k=KH))
        h0t_all = consts.tile([128, 2 * KH, BL], bf16, tag="h0t")
        nc.sync.dma_start(h0t_all[:],
                          h0t.rearrange("(d k p) b -> p (d k) b",
                                        p=128, k=KH))
        c0_sb = consts.tile([BL, 2, H], f32, tag="c0")
        nc.sync.dma_start(c0_sb[:], c0[:])
        bias_sb = [consts.tile([1, G4], bf16, tag=f"bias{d}",
                               name=f"bias{d}") for d in range(2)]
        for d in range(2):
            nc.sync.dma_start(bias_sb[d][:], bias_full[d:d + 1, :])
        # xg slice used for every step past the cut (== pure-bias row)
        bx_sb = [consts.tile([BL, G4], f32, tag=f"bx{d}", name=f"bx{d}")
                 for d in range(2)]

        # hidden history (transposed): hist[d][128, KH, S*BL], col = t*BL+b
        hist = []
        for d in range(2):
            ht = hist_pool.tile([128, KH, s_len * BL], bf16, tag=f"hist{d}",
                                name=f"hist{d}")
            hist.append(ht)
            nc.vector.memset(ht[:], 0.0)

        # ---------- phase 1: xg[d] = emb @ wih[d] + bias[d] ----------
        with ExitStack() as ph1:
            wp = ph1.enter_context(tc.tile_pool(name="wih", bufs=1))
            wih_all = wp.tile([128, 2 * KD, G4], bf16, tag="wih")
            nc.sync.dma_start(wih_all[:],
                              wih_full.rearrange("(d k p) g -> p (d k) g",
                                                 p=128, k=KD))
            ep = ph1.enter_context(tc.tile_pool(name="emb", bufs=3))
            etp = ph1.enter_context(tc.tile_pool(name="embT", bufs=3))
            xsp = ph1.enter_context(tc.tile_pool(name="xsb", bufs=3))
            pp = ph1.enter_context(
                tc.tile_pool(name="ph1ps", bufs=2, space="PSUM"))
            xp = ph1.enter_context(
                tc.tile_pool(name="xgps", bufs=3, space="PSUM"))
            for rt in range(NR):
                emb_sb = ep.tile([128, D], bf16, tag="emb")
                nc.sync.dma_start(emb_sb[:], emb[rt * 128:(rt + 1) * 128, :])
                etsb = etp.tile([128, D], bf16, tag="etsb")
                ps = pp.tile([128, D], bf16, tag="tps")
                for k in range(KD):
                    nc.tensor.transpose(ps[:, k * 128:(k + 1) * 128],
                                        emb_sb[:, k * 128:(k + 1) * 128],
                                        id128_sb[:])
                nc.vector.tensor_copy(etsb[:], ps[:])
                for d in range(2):
                    for n in range(3):
                        xps = xp.tile([128, 512], f32, tag="xps")
                        for k in range(KD):
                            nc.tensor.matmul(
                                xps[:],
                                etsb[:, k * 128:(k + 1) * 128],
                                wih_all[:, d * KD + k, n * 512:(n + 1) * 512],
                                start=(k == 0), stop=False)
                        nc.tensor.matmul(
                            xps[:], ones_sb[:],
                            bias_sb[d][:, n * 512:(n + 1) * 512],
                            start=False, stop=True)
                        xsb = xsp.tile([128, 512], f32, tag="xsb")
                        nc.vector.tensor_copy(xsb[:], xps[:])
                        nc.sync.dma_start(
                            xg_dram[d, rt * 128:(rt + 1) * 128,
                                    n * 512:(n + 1) * 512],
                            xsb[:])

        # bias-only xg rows for the tail steps
        for d in range(2):
            nc.sync.dma_start(
                bx_sb[d][:],
                xg_dram[d, (cut - 1) * BL:cut * BL, :])

        # ---------- phase 2: the two LSTM scans ----------
        with ExitStack() as ph2:
            xgp = ph2.enter_context(tc.tile_pool(name="xgin", bufs=2))
            gp = ph2.enter_context(
                tc.tile_pool(name="gps", bufs=2, space="PSUM"))
            htp = ph2.enter_context(
                tc.tile_pool(name="htps", bufs=2, space="PSUM"))
            sp = ph2.enter_context(tc.tile_pool(name="scan", bufs=2))
            cp = ph2.enter_context(tc.tile_pool(name="cbuf", bufs=2))

            for d in range(2):
                c_prev = c0_sb[:, d, :]
                xg_sb = None
                xg_lo = None
                for s in range(steps):
                    t_eff = s if d == 0 else s_len - 1 - s
                    if t_eff < cut:
                        # chunk-aligned DMA of XCH consecutive t slices
                        if d == 0:
                            lo = (s // XCH) * XCH
                            need = (s % XCH == 0)
                        else:
                            # t_eff descends cut-1 ... 0
                            j = cut - 1 - t_eff
                            lo = cut - 1 - (j // XCH) * XCH - (XCH - 1)
                            lo = max(lo, 0)
                            need = (j % XCH == 0)
                        if need:
                            nchunk = min(XCH, cut - lo)
                            xg_sb = xgp.tile([BL, XCH, G4], f32, tag="xg")
                            nc.sync.dma_start(
                                xg_sb[:, 0:nchunk, :],
                                xg_dram[d, lo * BL:(lo + nchunk) * BL, :]
                                .rearrange("(t b) g -> b t g", b=BL))
                            xg_lo = lo
                        xg_op = xg_sb[:, t_eff - xg_lo, :]
                    else:
                        xg_op = bx_sb[d][:]

                    if s == 0:
                        hT = [h0t_all[:, d * KH + k, :] for k in range(KH)]
                    else:
                        tp = t_eff - 1 if d == 0 else t_eff + 1
                        hT = [hist[d][:, k, tp * BL:(tp + 1) * BL]
                              for k in range(KH)]

                    g_ps = gp.tile([BL, G4], f32, tag="g")
                    for n in range(3):
                        for k in range(KH):
                            nc.tensor.matmul(
                                g_ps[:, n * 512:(n + 1) * 512],
                                hT[k],
                                whh_all[:, d * KH + k, n * 512:(n + 1) * 512],
                                start=(k == 0), stop=(k == KH - 1))
                    g_sb = sp.tile([BL, G4], f32, tag="gsb")
                    nc.vector.tensor_add(g_sb[:], g_ps[:], xg_op[:])
                    # gate order [i, f, o, g]
                    a_sb = sp.tile([BL, G4], f32, tag="asb")
                    nc.scalar.activation(a_sb[:, 0:3 * H], g_sb[:, 0:3 * H],
                                         AF.Sigmoid)
                    nc.scalar.activation(a_sb[:, 3 * H:G4], g_sb[:, 3 * H:G4],
                                         AF.Tanh)
                    c_new = cp.tile([BL, H], f32, tag="c")
                    tmp = sp.tile([BL, 2 * H], f32, tag="tmp")
                    nc.vector.tensor_mul(tmp[:, 0:H], a_sb[:, 0:H],
                                         a_sb[:, 3 * H:G4])      # i*tanh(g)
                    nc.vector.tensor_mul(c_new[:], a_sb[:, H:2 * H], c_prev)
                    nc.vector.tensor_add(c_new[:], c_new[:], tmp[:, 0:H])
                    nc.scalar.activation(tmp[:, H:2 * H], c_new[:], AF.Tanh)
                    h_sb = sp.tile([BL, H], f32, tag="h")
                    nc.vector.tensor_mul(h_sb[:], a_sb[:, 2 * H:3 * H],
                                         tmp[:, H:2 * H])
                    ht_ps = htp.tile([128, KH * BL], f32, tag="htps")
                    for k in range(KH):
                        nc.tensor.transpose(ht_ps[:, k * BL:(k + 1) * BL],
                                            h_sb[:, k * 128:(k + 1) * 128],
                                            id8_sb[:])
                    nc.vector.tensor_copy(
                        hist[d][:, :, t_eff * BL:(t_eff + 1) * BL],
                        ht_ps[:].rearrange("p (k b) -> p k b", k=KH))
                    c_prev = c_new[:]

        # ---------- phase 3: out = sum_d hist[d]^T @ wlin[d] ----------
        with ExitStack() as ph3:
            fp = ph3.enter_context(
                tc.tile_pool(name="fps", bufs=2, space="PSUM"))
            fsb = ph3.enter_context(tc.tile_pool(name="fsb", bufs=2))
            CW = 512 if (s_len * BL) % 512 == 0 else s_len * BL
            NCH = (s_len * BL) // CW
            for j in range(NCH):
                f_ps = fp.tile([T, CW], f32, tag="f")
                for d in range(2):
                    for k in range(KH):
                        nc.tensor.matmul(
                            f_ps[:],
                            wlin_all[:, d * KH + k, :],
                            hist[d][:, k, j * CW:(j + 1) * CW],
                            start=(d == 0 and k == 0),
                            stop=(d == 1 and k == KH - 1))
                f_sb = fsb.tile([T, CW], bf16, tag="fsb")
                nc.vector.tensor_copy(f_sb[:], f_ps[:])
                nc.sync.dma_start(out[:, j * CW:(j + 1) * CW], f_sb[:])

    nc.compile()
    # the PJRT lowering re-serializes the (immutable) module on every call
    # (~0.4s for this program); memoize it.
    _raw = [None]
    _orig = nc.to_json_bytes

    def _cached_json():
        if _raw[0] is None:
            _raw[0] = _orig()
        return _raw[0]

    nc.to_json_bytes = _cached_json
    return nc


def _get_program(cut, steps, s_len=S):
    key = (cut, steps, s_len)
    if key not in _cache:
        _cache[key] = _build_program(cut, steps, s_len)
    return _cache[key]


def _align_idx(start_ids, masks, cut):
    """Gather indices + keep mask for the first `cut` word slots."""
    sid = np.asarray(start_ids)
    msk = np.asarray(masks)
    Bb, Ss = sid.shape
    t = np.arange(cut)[None, :]
    n = (sid >= 0).sum(-1)
    last_sid = np.take_along_axis(sid, (n - 1)[:, None], axis=1)
    sid_c = sid[:, :cut]
    idx = np.where(t == 0, 0,
          np.where(t < n[:, None], sid_c - 1,
          np.where(t == n[:, None], last_sid, 0)))
    idx = np.clip(idx, 0, Ss - 1).astype(np.int64)
    sent_len = msk.sum(-1)
    keep = (t < sent_len[:, None])
    return idx, keep, int(sent_len.max())


def _host_prep(hidden_states, h0, c0, W_ih_f, W_hh_f, b_ih_f, b_hh_f,
               W_ih_b, W_hh_b, b_ih_b, b_hh_b, W_lin, b_lin,
               start_ids, masks, cut):
    import ml_dtypes
    bf16 = ml_dtypes.bfloat16

    hs = np.asarray(hidden_states, np.float32)
    idx, keep, _ = _align_idx(start_ids, masks, cut)
    gathered = np.take_along_axis(hs, idx[:, :, None], axis=1)  # [B,cut,D]
    gathered *= keep[:, :, None]
    emb_bf = gathered.astype(bf16)

    # replicated weights -> per-core 1/8 shards (views; concat copies later)
    def bfT(w, perm=None):
        w = np.asarray(w, np.float32).T
        if perm is not None:
            w = w[:, perm]
        return np.ascontiguousarray(w).astype(bf16)

    W_lin = np.asarray(W_lin, np.float32)
    wih_host = np.concatenate([bfT(W_ih_f, _PERM), bfT(W_ih_b, _PERM)], 0)
    whh_host = np.concatenate([bfT(W_hh_f, _PERM), bfT(W_hh_b, _PERM)], 0)
    wlin_host = np.concatenate(
        [np.ascontiguousarray(W_lin[:, :H].T).astype(bf16),
         np.ascontiguousarray(W_lin[:, H:].T).astype(bf16)], 0)
    bias_host = np.stack(
        [(np.asarray(b_ih_f, np.float32) + np.asarray(b_hh_f, np.float32))[_PERM],
         (np.asarray(b_ih_b, np.float32) + np.asarray(b_hh_b, np.float32))[_PERM]],
        0).astype(bf16)

    wih_shards = wih_host.reshape(NC, 2 * D // NC, G4)
    whh_shards = whh_host.reshape(NC, 2 * H // NC, G4)
    wlin_shards = wlin_host.reshape(NC, 2 * H // NC, T)
    bias_shards = bias_host.reshape(NC, 2 * G4 // NC)

    h0 = np.asarray(h0, np.float32)
    c0a = np.asarray(c0, np.float32)
    id128 = np.eye(128, dtype=bf16)
    id8 = np.eye(8, dtype=np.float32)

    in_maps = []
    for core in range(NC):
        bs = slice(core * BL, (core + 1) * BL)
        e = emb_bf[bs]                        # [BL, cut, D]
        e = np.ascontiguousarray(e.transpose(1, 0, 2)).reshape(-1, D)
        h0t = np.concatenate(
            [np.ascontiguousarray(h0[0, bs].T),
             np.ascontiguousarray(h0[1, bs].T)], 0).astype(bf16)  # [2H, BL]
        c0m = np.ascontiguousarray(
            np.stack([c0a[0, bs], c0a[1, bs]], 1))  # [BL, 2, H]
        in_maps.append({
            "emb": e,
            "h0t": h0t,
            "c0": c0m,
            "id128": id128,
            "id8": id8,
            "wih_sh": wih_shards[core],
            "whh_sh": whh_shards[core],
            "wlin_sh": wlin_shards[core],
            "bias_sh": bias_shards[core],
        })
    return in_maps


def kernel(hidden_states, h0, c0, W_ih_f, W_hh_f, b_ih_f, b_hh_f,
           W_ih_b, W_hh_b, b_ih_b, b_hh_b, W_lin, b_lin, start_ids, masks,
           _trace=False):
    _configure_jax_cache()
    from concourse.bass_utils import run_bass_kernel_spmd

    msk = np.asarray(masks)
    max_sent = int(msk.sum(-1).max())
    cut = CUT if max_sent <= CUT - 1 else S

    in_maps = _host_prep(
        hidden_states, h0, c0, W_ih_f, W_hh_f, b_ih_f, b_hh_f,
        W_ih_b, W_hh_b, b_ih_b, b_hh_b, W_lin, b_lin, start_ids, masks, cut)

    nc = _get_program(cut, STEPS)
    res = run_bass_kernel_spmd(nc, in_maps, list(range(NC)), trace=_trace)
    outs = res.results

    b_lin = np.asarray(b_lin, np.float32)
    feats = np.empty((B, S, T), np.float32)
    for core in range(NC):
        o = outs[core]["out"].astype(np.float32)       # [T, S*BL]
        o = o.reshape(T, S, BL).transpose(2, 1, 0)     # [BL, S, T]
        feats[core * BL:(core + 1) * BL] = o + b_lin
    if _trace:
        return feats, res
    return feats


# revision 8
# speedup vs baseline: 1.0290x; 1.0290x over previous
"""BERT-LSTM-CRF kernel for Trainium2, 8 NeuronCores.

Wall-clock-optimized: under the axon tunnel the end-to-end time of a
warm kernel() call is dominated by host->device input bytes and
per-call compile/lowering overhead, not device compute. Design:

  * Batch-shard 8-way (8 samples/core); each core runs BOTH LSTM
    directions, so the aligned embeddings ship once (not once per
    direction-core as in the 4+4 direction split).
  * Ragged cut: the word aligner zero-pads past sent_len (<= 258 for
    this generator), so embeds rows t >= CUT(=272) are all-zero and
    xg degenerates to the bias row. Only [CUT*8, 768] embedding rows
    ship per core; the scans reuse the xg slice at t=CUT-1 (pure
    bias) for every step past the cut. Falls back to a full-length
    program if masks ever exceed the cut.
  * bf16 wire format for embeddings + weights (matmuls in bf16 with
    fp32 PSUM accumulation; cell state stays fp32).
  * Replicated weights (W_ih/W_hh/W_lin/bias, both directions) are
    sharded 1/8th per core on the host and AllGathered on device over
    NeuronLink, cutting their upload 8x.
  * fwd+bwd LSTM + output projection fused on device; output is the
    per-core feats [22, S*8] in bf16; host adds b_lin.
  * jax persistent compilation cache + memoized BIR serialization so
    warm calls skip the per-call NEFF recompile that otherwise costs
    seconds inside run_bass_kernel_spmd's fresh-jit path.

Per-call upload ~36MB vs ~305MB for the direction-split fp32 version.
"""
import os
import sys
import tempfile
import numpy as np

sys.path.insert(0, "/opt/trn_rl_repo")

B, S, D, H, T = 64, 512, 768, 384, 22
G4 = 4 * H            # 1536 gate rows
BL = 8                # batch per core
NC = 8
KD = D // 128         # 6 contraction chunks of the input GEMM
KH = H // 128         # 3 hidden chunks
CUT = 272             # compile-time ragged cut (>= max sent_len+1, x16)
STEPS = int(os.environ.get("KSTEPS", str(S)))
XCH = 4               # scan timesteps per xg DMA chunk

_cache = {}
_cfg_done = [False]

# gate-order permutation: torch [i,f,g,o] -> kernel [i,f,o,g]
_PERM = np.concatenate([np.arange(0, H), np.arange(H, 2 * H),
                        np.arange(3 * H, 4 * H), np.arange(2 * H, 3 * H)])


def _configure_jax_cache():
    """Persistent XLA compilation cache: the runner rebuilds its jit
    closure every call, so without this every warm call re-runs the
    multi-second NEFF compile."""
    if _cfg_done[0]:
        return
    _cfg_done[0] = True
    try:
        import jax
        jax.config.update(
            "jax_compilation_cache_dir",
            os.path.join(tempfile.gettempdir(), "jax_comp_cache"))
        jax.config.update("jax_persistent_cache_min_entry_size_bytes", -1)
        jax.config.update("jax_persistent_cache_min_compile_time_secs", 0.0)
    except Exception:
        pass


def _build_program(cut, steps, s_len=S):
    from concourse import bacc, tile, mybir
    from contextlib import ExitStack

    f32 = mybir.dt.float32
    bf16 = mybir.dt.bfloat16
    AF = mybir.ActivationFunctionType

    nc = bacc.Bacc("TRN2", target_bir_lowering=False, debug=False,
                   num_devices=NC)

    NR = (BL * cut) // 128   # emb row tiles

    # all inputs packed into two blobs: each extra jit argument costs
    # ~80ms of per-call transfer overhead through the axon tunnel
    SZ = {"emb": BL * cut * D, "h0t": 2 * H * BL, "id128": 128 * 128,
          "wih_sh": (2 * D // NC) * G4, "whh_sh": (2 * H // NC) * G4,
          "wlin_sh": (2 * H // NC) * T, "bias_sh": 2 * G4 // NC}
    OFF = {}
    _o = 0
    for _k in ("emb", "h0t", "id128", "wih_sh", "whh_sh", "wlin_sh",
               "bias_sh"):
        OFF[_k] = _o
        _o += SZ[_k]
    pbf = nc.dram_tensor("packed_bf", [_o], bf16, kind="ExternalInput")
    pf32 = nc.dram_tensor("packed_f32", [BL * 2 * H + 64], f32,
                          kind="ExternalInput")

    def seg(name):
        return pbf[OFF[name]:OFF[name] + SZ[name]]
    wih_full = nc.dram_tensor("wih_full", [2 * D, G4], bf16,
                              addr_space="Shared")
    whh_full = nc.dram_tensor("whh_full", [2 * H, G4], bf16,
                              addr_space="Shared")
    wlin_full = nc.dram_tensor("wlin_full", [2 * H, T], bf16,
                               addr_space="Shared")
    bias_full = nc.dram_tensor("bias_full", [2, G4], bf16,
                               addr_space="Shared")
    out = nc.dram_tensor("out", [T, s_len * BL], bf16, kind="ExternalOutput")
    xg_dram = nc.dram_tensor("xg_scratch", [2, cut * BL, G4], f32)
    # collectives cannot read IO tensors: bounce shards through internal dram
    wih_bn = nc.dram_tensor("wih_bn", [(2 * D // NC) * G4], bf16)
    whh_bn = nc.dram_tensor("whh_bn", [(2 * H // NC) * G4], bf16)
    wlin_bn = nc.dram_tensor("wlin_bn", [(2 * H // NC) * T], bf16)
    bias_bn = nc.dram_tensor("bias_bn", [2 * G4 // NC], bf16)

    grp = [list(range(NC))]

    with tile.TileContext(nc) as tc, ExitStack() as big:
        for sname, bn, full in (("wih_sh", wih_bn, wih_full),
                                ("whh_sh", whh_bn, whh_full),
                                ("wlin_sh", wlin_bn, wlin_full),
                                ("bias_sh", bias_bn, bias_full)):
            nc.sync.dma_start(bn[:], seg(sname))
            nc.gpsimd.collective_compute(
                "AllGather", mybir.AluOpType.bypass, replica_groups=grp,
                ins=[bn[:]], outs=[full[:]])

        consts = big.enter_context(tc.tile_pool(name="consts", bufs=1))
        hist_pool = big.enter_context(tc.tile_pool(name="hist", bufs=1))

        id128_sb = consts.tile([128, 128], bf16, tag="id128")
        nc.sync.dma_start(id128_sb[:],
                          seg("id128").rearrange("(a b) -> a b", a=128))
        id8_sb = consts.tile([8, 8], f32, tag="id8")
        nc.sync.dma_start(id8_sb[:],
                          pf32[BL * 2 * H:].rearrange("(a b) -> a b", a=8))
        ones_sb = consts.tile([1, 128], bf16, tag="ones")
        nc.vector.memset(ones_sb[:], 1.0)

        whh_all = consts.tile([128, 2 * KH, G4], bf16, tag="whh")
        nc.sync.dma_start(whh_all[:],
                          whh_full.rearrange("(d k p) g -> p (d k) g",
                                             p=128, k=KH))
        wlin_all = consts.tile([128, 2 * KH, T], bf16, tag="wlin")
        nc.sync.dma_start(wlin_all[:],
                          wlin_full.rearrange("(d k p) t -> p (d k) t",
                                              p=128, k=KH))
        h0t_all = consts.tile([128, 2 * KH, BL], bf16, tag="h0t")
        nc.sync.dma_start(h0t_all[:],
                          seg("h0t").rearrange("(d k p b) -> p (d k) b",
                                               p=128, k=KH, b=BL))
        c0_sb = consts.tile([BL, 2, H], f32, tag="c0")
        nc.sync.dma_start(c0_sb[:],
                          pf32[0:BL * 2 * H].rearrange("(b d h) -> b d h",
                                                       b=BL, d=2))
        bias_sb = [consts.tile([1, G4], bf16, tag=f"bias{d}",
                               name=f"bias{d}") for d in range(2)]
        for d in range(2):
            nc.sync.dma_start(bias_sb[d][:], bias_full[d:d + 1, :])
        # xg slice used for every step past the cut (== pure-bias row)
        bx_sb = [consts.tile([BL, G4], f32, tag=f"bx{d}", name=f"bx{d}")
                 for d in range(2)]

        # hidden history (transposed): hist[d][128, KH, S*BL], col = t*BL+b
        hist = []
        for d in range(2):
            ht = hist_pool.tile([128, KH, s_len * BL], bf16, tag=f"hist{d}",
                                name=f"hist{d}")
            hist.append(ht)
            nc.vector.memset(ht[:], 0.0)

        # ---------- phase 1: xg[d] = emb @ wih[d] + bias[d] ----------
        with ExitStack() as ph1:
            wp = ph1.enter_context(tc.tile_pool(name="wih", bufs=1))
            wih_all = wp.tile([128, 2 * KD, G4], bf16, tag="wih")
            nc.sync.dma_start(wih_all[:],
                              wih_full.rearrange("(d k p) g -> p (d k) g",
                                                 p=128, k=KD))
            ep = ph1.enter_context(tc.tile_pool(name="emb", bufs=3))
            etp = ph1.enter_context(tc.tile_pool(name="embT", bufs=3))
            xsp = ph1.enter_context(tc.tile_pool(name="xsb", bufs=3))
            pp = ph1.enter_context(
                tc.tile_pool(name="ph1ps", bufs=2, space="PSUM"))
            xp = ph1.enter_context(
                tc.tile_pool(name="xgps", bufs=3, space="PSUM"))
            for rt in range(NR):
                emb_sb = ep.tile([128, D], bf16, tag="emb")
                nc.sync.dma_start(
                    emb_sb[:],
                    pbf[rt * 128 * D:(rt + 1) * 128 * D]
                    .rearrange("(p d) -> p d", p=128))
                etsb = etp.tile([128, D], bf16, tag="etsb")
                ps = pp.tile([128, D], bf16, tag="tps")
                for k in range(KD):
                    nc.tensor.transpose(ps[:, k * 128:(k + 1) * 128],
                                        emb_sb[:, k * 128:(k + 1) * 128],
                                        id128_sb[:])
                nc.vector.tensor_copy(etsb[:], ps[:])
                for d in range(2):
                    for n in range(3):
                        xps = xp.tile([128, 512], f32, tag="xps")
                        for k in range(KD):
                            nc.tensor.matmul(
                                xps[:],
                                etsb[:, k * 128:(k + 1) * 128],
                                wih_all[:, d * KD + k, n * 512:(n + 1) * 512],
                                start=(k == 0), stop=False)
                        nc.tensor.matmul(
                            xps[:], ones_sb[:],
                            bias_sb[d][:, n * 512:(n + 1) * 512],
                            start=False, stop=True)
                        xsb = xsp.tile([128, 512], f32, tag="xsb")
                        nc.vector.tensor_copy(xsb[:], xps[:])
                        nc.sync.dma_start(
                            xg_dram[d, rt * 128:(rt + 1) * 128,
                                    n * 512:(n + 1) * 512],
                            xsb[:])

        # bias-only xg rows for the tail steps
        for d in range(2):
            nc.sync.dma_start(
                bx_sb[d][:],
                xg_dram[d, (cut - 1) * BL:cut * BL, :])

        # ---------- phase 2: the two LSTM scans ----------
        with ExitStack() as ph2:
            xgp = ph2.enter_context(tc.tile_pool(name="xgin", bufs=2))
            gp = ph2.enter_context(
                tc.tile_pool(name="gps", bufs=2, space="PSUM"))
            htp = ph2.enter_context(
                tc.tile_pool(name="htps", bufs=2, space="PSUM"))
            sp = ph2.enter_context(tc.tile_pool(name="scan", bufs=2))
            cp = ph2.enter_context(tc.tile_pool(name="cbuf", bufs=2))

            for d in range(2):
                c_prev = c0_sb[:, d, :]
                xg_sb = None
                xg_lo = None
                for s in range(steps):
                    t_eff = s if d == 0 else s_len - 1 - s
                    if t_eff < cut:
                        # chunk-aligned DMA of XCH consecutive t slices
                        if d == 0:
                            lo = (s // XCH) * XCH
                            need = (s % XCH == 0)
                        else:
                            # t_eff descends cut-1 ... 0
                            j = cut - 1 - t_eff
                            lo = cut - 1 - (j // XCH) * XCH - (XCH - 1)
                            lo = max(lo, 0)
                            need = (j % XCH == 0)
                        if need:
                            nchunk = min(XCH, cut - lo)
                            xg_sb = xgp.tile([BL, XCH, G4], f32, tag="xg")
                            nc.sync.dma_start(
                                xg_sb[:, 0:nchunk, :],
                                xg_dram[d, lo * BL:(lo + nchunk) * BL, :]
                                .rearrange("(t b) g -> b t g", b=BL))
                            xg_lo = lo
                        xg_op = xg_sb[:, t_eff - xg_lo, :]
                    else:
                        xg_op = bx_sb[d][:]

                    if s == 0:
                        hT = [h0t_all[:, d * KH + k, :] for k in range(KH)]
                    else:
                        tp = t_eff - 1 if d == 0 else t_eff + 1
                        hT = [hist[d][:, k, tp * BL:(tp + 1) * BL]
                              for k in range(KH)]

                    g_ps = gp.tile([BL, G4], f32, tag="g")
                    for n in range(3):
                        for k in range(KH):
                            nc.tensor.matmul(
                                g_ps[:, n * 512:(n + 1) * 512],
                                hT[k],
                                whh_all[:, d * KH + k, n * 512:(n + 1) * 512],
                                start=(k == 0), stop=(k == KH - 1))
                    g_sb = sp.tile([BL, G4], f32, tag="gsb")
                    nc.vector.tensor_add(g_sb[:], g_ps[:], xg_op[:])
                    # gate order [i, f, o, g]
                    a_sb = sp.tile([BL, G4], f32, tag="asb")
                    nc.scalar.activation(a_sb[:, 0:3 * H], g_sb[:, 0:3 * H],
                                         AF.Sigmoid)
                    nc.scalar.activation(a_sb[:, 3 * H:G4], g_sb[:, 3 * H:G4],
                                         AF.Tanh)
                    c_new = cp.tile([BL, H], f32, tag="c")
                    tmp = sp.tile([BL, 2 * H], f32, tag="tmp")
                    nc.vector.tensor_mul(tmp[:, 0:H], a_sb[:, 0:H],
                                         a_sb[:, 3 * H:G4])      # i*tanh(g)
                    nc.vector.tensor_mul(c_new[:], a_sb[:, H:2 * H], c_prev)
                    nc.vector.tensor_add(c_new[:], c_new[:], tmp[:, 0:H])
                    nc.scalar.activation(tmp[:, H:2 * H], c_new[:], AF.Tanh)
                    h_sb = sp.tile([BL, H], f32, tag="h")
                    nc.vector.tensor_mul(h_sb[:], a_sb[:, 2 * H:3 * H],
                                         tmp[:, H:2 * H])
                    ht_ps = htp.tile([128, KH * BL], f32, tag="htps")
                    for k in range(KH):
                        nc.tensor.transpose(ht_ps[:, k * BL:(k + 1) * BL],
                                            h_sb[:, k * 128:(k + 1) * 128],
                                            id8_sb[:])
                    nc.vector.tensor_copy(
                        hist[d][:, :, t_eff * BL:(t_eff + 1) * BL],
                        ht_ps[:].rearrange("p (k b) -> p k b", k=KH))
                    c_prev = c_new[:]

        # ---------- phase 3: out = sum_d hist[d]^T @ wlin[d] ----------
        with ExitStack() as ph3:
            fp = ph3.enter_context(
                tc.tile_pool(name="fps", bufs=2, space="PSUM"))
            fsb = ph3.enter_context(tc.tile_pool(name="fsb", bufs=2))
            CW = 512 if (s_len * BL) % 512 == 0 else s_len * BL
            NCH = (s_len * BL) // CW
            for j in range(NCH):
                f_ps = fp.tile([T, CW], f32, tag="f")
                for d in range(2):
                    for k in range(KH):
                        nc.tensor.matmul(
                            f_ps[:],
                            wlin_all[:, d * KH + k, :],
                            hist[d][:, k, j * CW:(j + 1) * CW],
                            start=(d == 0 and k == 0),
                            stop=(d == 1 and k == KH - 1))
                f_sb = fsb.tile([T, CW], bf16, tag="fsb")
                nc.vector.tensor_copy(f_sb[:], f_ps[:])
                nc.sync.dma_start(out[:, j * CW:(j + 1) * CW], f_sb[:])

    nc.compile()
    # the PJRT lowering re-serializes the (immutable) module on every call
    # (~0.4s for this program); memoize it.
    _raw = [None]
    _orig = nc.to_json_bytes

    def _cached_json():
        if _raw[0] is None:
            _raw[0] = _orig()
        return _raw[0]

    nc.to_json_bytes = _cached_json
    return nc


def _get_program(cut, steps, s_len=S):
    key = (cut, steps, s_len)
    if key not in _cache:
        _cache[key] = _build_program(cut, steps, s_len)
    return _cache[key]


def _align_idx(start_ids, masks, cut):
    """Gather indices + keep mask for the first `cut` word slots."""
    sid = np.asarray(start_ids)
    msk = np.asarray(masks)
    Bb, Ss = sid.shape
    t = np.arange(cut)[None, :]
    n = (sid >= 0).sum(-1)
    last_sid = np.take_along_axis(sid, (n - 1)[:, None], axis=1)
    sid_c = sid[:, :cut]
    idx = np.where(t == 0, 0,
          np.where(t < n[:, None], sid_c - 1,
          np.where(t == n[:, None], last_sid, 0)))
    idx = np.clip(idx, 0, Ss - 1).astype(np.int64)
    sent_len = msk.sum(-1)
    keep = (t < sent_len[:, None])
    return idx, keep, int(sent_len.max())


def _host_prep(hidden_states, h0, c0, W_ih_f, W_hh_f, b_ih_f, b_hh_f,
               W_ih_b, W_hh_b, b_ih_b, b_hh_b, W_lin, b_lin,
               start_ids, masks, cut):
    import ml_dtypes
    bf16 = ml_dtypes.bfloat16

    hs = np.asarray(hidden_states, np.float32)
    idx, keep, _ = _align_idx(start_ids, masks, cut)
    gathered = np.take_along_axis(hs, idx[:, :, None], axis=1)  # [B,cut,D]
    gathered *= keep[:, :, None]
    emb_bf = gathered.astype(bf16)

    # replicated weights -> per-core 1/8 shards (views; concat copies later)
    def bfT(w, perm=None):
        w = np.asarray(w, np.float32).T
        if perm is not None:
            w = w[:, perm]
        return np.ascontiguousarray(w).astype(bf16)

    W_lin = np.asarray(W_lin, np.float32)
    wih_host = np.concatenate([bfT(W_ih_f, _PERM), bfT(W_ih_b, _PERM)], 0)
    whh_host = np.concatenate([bfT(W_hh_f, _PERM), bfT(W_hh_b, _PERM)], 0)
    wlin_host = np.concatenate(
        [np.ascontiguousarray(W_lin[:, :H].T).astype(bf16),
         np.ascontiguousarray(W_lin[:, H:].T).astype(bf16)], 0)
    bias_host = np.stack(
        [(np.asarray(b_ih_f, np.float32) + np.asarray(b_hh_f, np.float32))[_PERM],
         (np.asarray(b_ih_b, np.float32) + np.asarray(b_hh_b, np.float32))[_PERM]],
        0).astype(bf16)

    wih_shards = wih_host.reshape(NC, 2 * D // NC, G4)
    whh_shards = whh_host.reshape(NC, 2 * H // NC, G4)
    wlin_shards = wlin_host.reshape(NC, 2 * H // NC, T)
    bias_shards = bias_host.reshape(NC, 2 * G4 // NC)

    h0 = np.asarray(h0, np.float32)
    c0a = np.asarray(c0, np.float32)
    id128 = np.eye(128, dtype=bf16)
    id8 = np.eye(8, dtype=np.float32)

    in_maps = []
    for core in range(NC):
        bs = slice(core * BL, (core + 1) * BL)
        e = emb_bf[bs]                        # [BL, cut, D]
        e = np.ascontiguousarray(e.transpose(1, 0, 2)).reshape(-1, D)
        h0t = np.concatenate(
            [np.ascontiguousarray(h0[0, bs].T),
             np.ascontiguousarray(h0[1, bs].T)], 0).astype(bf16)  # [2H, BL]
        c0m = np.ascontiguousarray(
            np.stack([c0a[0, bs], c0a[1, bs]], 1))  # [BL, 2, H]
        in_maps.append({
            "packed_bf": np.concatenate(
                [e.ravel(), h0t.ravel(), id128.ravel(),
                 wih_shards[core].ravel(), whh_shards[core].ravel(),
                 wlin_shards[core].ravel(), bias_shards[core].ravel()]),
            "packed_f32": np.concatenate([c0m.ravel(), id8.ravel()]),
        })
    return in_maps


def kernel(hidden_states, h0, c0, W_ih_f, W_hh_f, b_ih_f, b_hh_f,
           W_ih_b, W_hh_b, b_ih_b, b_hh_b, W_lin, b_lin, start_ids, masks,
           _trace=False):
    _configure_jax_cache()
    from concourse.bass_utils import run_bass_kernel_spmd

    msk = np.asarray(masks)
    max_sent = int(msk.sum(-1).max())
    cut = CUT if max_sent <= CUT - 1 else S

    in_maps = _host_prep(
        hidden_states, h0, c0, W_ih_f, W_hh_f, b_ih_f, b_hh_f,
        W_ih_b, W_hh_b, b_ih_b, b_hh_b, W_lin, b_lin, start_ids, masks, cut)

    nc = _get_program(cut, STEPS)
    res = run_bass_kernel_spmd(nc, in_maps, list(range(NC)), trace=_trace)
    outs = res.results

    b_lin = np.asarray(b_lin, np.float32)
    feats = np.empty((B, S, T), np.float32)
    for core in range(NC):
        o = outs[core]["out"].astype(np.float32)       # [T, S*BL]
        o = o.reshape(T, S, BL).transpose(2, 1, 0)     # [BL, S, T]
        feats[core * BL:(core + 1) * BL] = o + b_lin
    if _trace:
        return feats, res
    return feats


# revision 9
# speedup vs baseline: 1.0648x; 1.0349x over previous
"""BERT-LSTM-CRF kernel for Trainium2, 8 NeuronCores.

Wall-clock-optimized: under the axon tunnel the end-to-end time of a
warm kernel() call is dominated by host->device input bytes and
per-call compile/lowering overhead, not device compute. Design:

  * Batch-shard 8-way (8 samples/core); each core runs BOTH LSTM
    directions, so the aligned embeddings ship once (not once per
    direction-core as in the 4+4 direction split).
  * Ragged cut: the word aligner zero-pads past sent_len (<= 258 for
    this generator), so embeds rows t >= CUT(=272) are all-zero and
    xg degenerates to the bias row. Only [CUT*8, 768] embedding rows
    ship per core; the scans reuse the xg slice at t=CUT-1 (pure
    bias) for every step past the cut. Falls back to a full-length
    program if masks ever exceed the cut.
  * bf16 wire format for embeddings + weights (matmuls in bf16 with
    fp32 PSUM accumulation; cell state stays fp32).
  * Replicated weights (W_ih/W_hh/W_lin/bias, both directions) are
    sharded 1/8th per core on the host and AllGathered on device over
    NeuronLink, cutting their upload 8x.
  * fwd+bwd LSTM + output projection fused on device; output is the
    per-core feats [22, S*8] in bf16; host adds b_lin.
  * jax persistent compilation cache + memoized BIR serialization so
    warm calls skip the per-call NEFF recompile that otherwise costs
    seconds inside run_bass_kernel_spmd's fresh-jit path.

Per-call upload ~36MB vs ~305MB for the direction-split fp32 version.
"""
import os
import sys
import tempfile
import numpy as np

sys.path.insert(0, "/opt/trn_rl_repo")

B, S, D, H, T = 64, 512, 768, 384, 22
G4 = 4 * H            # 1536 gate rows
BL = 8                # batch per core
NC = 8
KD = D // 128         # 6 contraction chunks of the input GEMM
KH = H // 128         # 3 hidden chunks
CUT = 272             # compile-time ragged cut (>= max sent_len+1, x16)
STEPS = int(os.environ.get("KSTEPS", str(S)))
XCH = 4               # scan timesteps per xg DMA chunk

_cache = {}
_cfg_done = [False]

# gate-order permutation: torch [i,f,g,o] -> kernel [i,f,o,g]
_PERM = np.concatenate([np.arange(0, H), np.arange(H, 2 * H),
                        np.arange(3 * H, 4 * H), np.arange(2 * H, 3 * H)])


def _configure_jax_cache():
    """Persistent XLA compilation cache: the runner rebuilds its jit
    closure every call, so without this every warm call re-runs the
    multi-second NEFF compile."""
    if _cfg_done[0]:
        return
    _cfg_done[0] = True
    try:
        import jax
        jax.config.update(
            "jax_compilation_cache_dir",
            os.path.join(tempfile.gettempdir(), "jax_comp_cache"))
        jax.config.update("jax_persistent_cache_min_entry_size_bytes", -1)
        jax.config.update("jax_persistent_cache_min_compile_time_secs", 0.0)
    except Exception:
        pass


def _build_program(cut, steps, s_len=S):
    from concourse import bacc, tile, mybir
    from contextlib import ExitStack

    f32 = mybir.dt.float32
    bf16 = mybir.dt.bfloat16
    AF = mybir.ActivationFunctionType

    nc = bacc.Bacc("TRN2", target_bir_lowering=False, debug=False,
                   num_devices=NC)

    NR = (BL * cut) // 128   # emb row tiles

    # all inputs packed into two blobs: each extra jit argument costs
    # ~80ms of per-call transfer overhead through the axon tunnel
    SZ = {"emb": BL * cut * D, "h0t": 2 * H * BL, "id128": 128 * 128,
          "wih_sh": (2 * D // NC) * G4, "whh_sh": (2 * H // NC) * G4,
          "wlin_sh": (2 * H // NC) * T, "bias_sh": 2 * G4 // NC}
    OFF = {}
    _o = 0
    for _k in ("emb", "h0t", "id128", "wih_sh", "whh_sh", "wlin_sh",
               "bias_sh"):
        OFF[_k] = _o
        _o += SZ[_k]
    pbf = nc.dram_tensor("packed_bf", [_o], bf16, kind="ExternalInput")
    pf32 = nc.dram_tensor("packed_f32", [BL * 2 * H + 64], f32,
                          kind="ExternalInput")

    def seg(name):
        return pbf[OFF[name]:OFF[name] + SZ[name]]
    wih_full = nc.dram_tensor("wih_full", [2 * D, G4], bf16,
                              addr_space="Shared")
    whh_full = nc.dram_tensor("whh_full", [2 * H, G4], bf16,
                              addr_space="Shared")
    wlin_full = nc.dram_tensor("wlin_full", [2 * H, T], bf16,
                               addr_space="Shared")
    bias_full = nc.dram_tensor("bias_full", [2, G4], bf16,
                               addr_space="Shared")
    out = nc.dram_tensor("out", [T, s_len * BL], bf16, kind="ExternalOutput")
    xg_dram = nc.dram_tensor("xg_scratch", [2, cut * BL, G4], f32)
    # collectives cannot read IO tensors: bounce shards through internal dram
    wih_bn = nc.dram_tensor("wih_bn", [(2 * D // NC) * G4], bf16)
    whh_bn = nc.dram_tensor("whh_bn", [(2 * H // NC) * G4], bf16)
    wlin_bn = nc.dram_tensor("wlin_bn", [(2 * H // NC) * T], bf16)
    bias_bn = nc.dram_tensor("bias_bn", [2 * G4 // NC], bf16)

    grp = [list(range(NC))]

    with tile.TileContext(nc) as tc, ExitStack() as big:
        for sname, bn, full in (("wih_sh", wih_bn, wih_full),
                                ("whh_sh", whh_bn, whh_full),
                                ("wlin_sh", wlin_bn, wlin_full),
                                ("bias_sh", bias_bn, bias_full)):
            nc.sync.dma_start(bn[:], seg(sname))
            nc.gpsimd.collective_compute(
                "AllGather", mybir.AluOpType.bypass, replica_groups=grp,
                ins=[bn[:]], outs=[full[:]])

        consts = big.enter_context(tc.tile_pool(name="consts", bufs=1))
        hist_pool = big.enter_context(tc.tile_pool(name="hist", bufs=1))

        id128_sb = consts.tile([128, 128], bf16, tag="id128")
        nc.sync.dma_start(id128_sb[:],
                          seg("id128").rearrange("(a b) -> a b", a=128))
        id8_sb = consts.tile([8, 8], f32, tag="id8")
        nc.sync.dma_start(id8_sb[:],
                          pf32[BL * 2 * H:].rearrange("(a b) -> a b", a=8))
        ones_sb = consts.tile([1, 128], bf16, tag="ones")
        nc.vector.memset(ones_sb[:], 1.0)

        whh_all = consts.tile([128, 2 * KH, G4], bf16, tag="whh")
        nc.sync.dma_start(whh_all[:],
                          whh_full.rearrange("(d k p) g -> p (d k) g",
                                             p=128, k=KH))
        wlin_all = consts.tile([128, 2 * KH, T], bf16, tag="wlin")
        nc.sync.dma_start(wlin_all[:],
                          wlin_full.rearrange("(d k p) t -> p (d k) t",
                                              p=128, k=KH))
        h0t_all = consts.tile([128, 2 * KH, BL], bf16, tag="h0t")
        nc.sync.dma_start(h0t_all[:],
                          seg("h0t").rearrange("(d k p b) -> p (d k) b",
                                               p=128, k=KH, b=BL))
        c0_sb = consts.tile([BL, 2, H], f32, tag="c0")
        nc.sync.dma_start(c0_sb[:],
                          pf32[0:BL * 2 * H].rearrange("(b d h) -> b d h",
                                                       b=BL, d=2))
        bias_sb = [consts.tile([1, G4], bf16, tag=f"bias{d}",
                               name=f"bias{d}") for d in range(2)]
        for d in range(2):
            nc.sync.dma_start(bias_sb[d][:], bias_full[d:d + 1, :])
        # xg slice used for every step past the cut (== pure-bias row)
        bx_sb = [consts.tile([BL, G4], f32, tag=f"bx{d}", name=f"bx{d}")
                 for d in range(2)]

        # hidden history (transposed): hist[d][128, KH, S*BL], col = t*BL+b
        hist = []
        for d in range(2):
            ht = hist_pool.tile([128, KH, s_len * BL], bf16, tag=f"hist{d}",
                                name=f"hist{d}")
            hist.append(ht)
            nc.vector.memset(ht[:], 0.0)

        # ---------- phase 1: xg[d] = emb @ wih[d] + bias[d] ----------
        with ExitStack() as ph1:
            wp = ph1.enter_context(tc.tile_pool(name="wih", bufs=1))
            wih_all = wp.tile([128, 2 * KD, G4], bf16, tag="wih")
            nc.sync.dma_start(wih_all[:],
                              wih_full.rearrange("(d k p) g -> p (d k) g",
                                                 p=128, k=KD))
            ep = ph1.enter_context(tc.tile_pool(name="emb", bufs=3))
            etp = ph1.enter_context(tc.tile_pool(name="embT", bufs=3))
            xsp = ph1.enter_context(tc.tile_pool(name="xsb", bufs=3))
            pp = ph1.enter_context(
                tc.tile_pool(name="ph1ps", bufs=2, space="PSUM"))
            xp = ph1.enter_context(
                tc.tile_pool(name="xgps", bufs=3, space="PSUM"))
            for rt in range(NR):
                emb_sb = ep.tile([128, D], bf16, tag="emb")
                nc.sync.dma_start(
                    emb_sb[:],
                    pbf[rt * 128 * D:(rt + 1) * 128 * D]
                    .rearrange("(p d) -> p d", p=128))
                etsb = etp.tile([128, D], bf16, tag="etsb")
                ps = pp.tile([128, D], bf16, tag="tps")
                for k in range(KD):
                    nc.tensor.transpose(ps[:, k * 128:(k + 1) * 128],
                                        emb_sb[:, k * 128:(k + 1) * 128],
                                        id128_sb[:])
                nc.vector.tensor_copy(etsb[:], ps[:])
                for d in range(2):
                    for n in range(3):
                        xps = xp.tile([128, 512], f32, tag="xps")
                        for k in range(KD):
                            nc.tensor.matmul(
                                xps[:],
                                etsb[:, k * 128:(k + 1) * 128],
                                wih_all[:, d * KD + k, n * 512:(n + 1) * 512],
                                start=(k == 0), stop=False)
                        nc.tensor.matmul(
                            xps[:], ones_sb[:],
                            bias_sb[d][:, n * 512:(n + 1) * 512],
                            start=False, stop=True)
                        xsb = xsp.tile([128, 512], f32, tag="xsb")
                        nc.vector.tensor_copy(xsb[:], xps[:])
                        nc.sync.dma_start(
                            xg_dram[d, rt * 128:(rt + 1) * 128,
                                    n * 512:(n + 1) * 512],
                            xsb[:])

        # bias-only xg rows for the tail steps
        for d in range(2):
            nc.sync.dma_start(
                bx_sb[d][:],
                xg_dram[d, (cut - 1) * BL:cut * BL, :])

        # ---------- phase 2: the two LSTM scans ----------
        with ExitStack() as ph2:
            xgp = ph2.enter_context(tc.tile_pool(name="xgin", bufs=2))
            gp = ph2.enter_context(
                tc.tile_pool(name="gps", bufs=2, space="PSUM"))
            htp = ph2.enter_context(
                tc.tile_pool(name="htps", bufs=2, space="PSUM"))
            sp = ph2.enter_context(tc.tile_pool(name="scan", bufs=2))
            cp = ph2.enter_context(tc.tile_pool(name="cbuf", bufs=2))

            for d in range(2):
                c_prev = c0_sb[:, d, :]
                xg_sb = None
                xg_lo = None
                for s in range(steps):
                    t_eff = s if d == 0 else s_len - 1 - s
                    if t_eff < cut:
                        # chunk-aligned DMA of XCH consecutive t slices
                        if d == 0:
                            lo = (s // XCH) * XCH
                            need = (s % XCH == 0)
                        else:
                            # t_eff descends cut-1 ... 0
                            j = cut - 1 - t_eff
                            lo = cut - 1 - (j // XCH) * XCH - (XCH - 1)
                            lo = max(lo, 0)
                            need = (j % XCH == 0)
                        if need:
                            nchunk = min(XCH, cut - lo)
                            xg_sb = xgp.tile([BL, XCH, G4], f32, tag="xg")
                            nc.sync.dma_start(
                                xg_sb[:, 0:nchunk, :],
                                xg_dram[d, lo * BL:(lo + nchunk) * BL, :]
                                .rearrange("(t b) g -> b t g", b=BL))
                            xg_lo = lo
                        xg_op = xg_sb[:, t_eff - xg_lo, :]
                    else:
                        xg_op = bx_sb[d][:]

                    if s == 0:
                        hT = [h0t_all[:, d * KH + k, :] for k in range(KH)]
                    else:
                        tp = t_eff - 1 if d == 0 else t_eff + 1
                        hT = [hist[d][:, k, tp * BL:(tp + 1) * BL]
                              for k in range(KH)]

                    g_ps = gp.tile([BL, G4], f32, tag="g")
                    for n in range(3):
                        for k in range(KH):
                            nc.tensor.matmul(
                                g_ps[:, n * 512:(n + 1) * 512],
                                hT[k],
                                whh_all[:, d * KH + k, n * 512:(n + 1) * 512],
                                start=(k == 0), stop=(k == KH - 1))
                    g_sb = sp.tile([BL, G4], f32, tag="gsb")
                    nc.vector.tensor_add(g_sb[:], g_ps[:], xg_op[:])
                    # gate order [i, f, o, g]
                    a_sb = sp.tile([BL, G4], f32, tag="asb")
                    nc.scalar.activation(a_sb[:, 0:3 * H], g_sb[:, 0:3 * H],
                                         AF.Sigmoid)
                    nc.scalar.activation(a_sb[:, 3 * H:G4], g_sb[:, 3 * H:G4],
                                         AF.Tanh)
                    c_new = cp.tile([BL, H], f32, tag="c")
                    tmp = sp.tile([BL, 2 * H], f32, tag="tmp")
                    nc.vector.tensor_mul(tmp[:, 0:H], a_sb[:, 0:H],
                                         a_sb[:, 3 * H:G4])      # i*tanh(g)
                    nc.vector.tensor_mul(c_new[:], a_sb[:, H:2 * H], c_prev)
                    nc.vector.tensor_add(c_new[:], c_new[:], tmp[:, 0:H])
                    nc.scalar.activation(tmp[:, H:2 * H], c_new[:], AF.Tanh)
                    h_sb = sp.tile([BL, H], f32, tag="h")
                    nc.vector.tensor_mul(h_sb[:], a_sb[:, 2 * H:3 * H],
                                         tmp[:, H:2 * H])
                    ht_ps = htp.tile([128, KH * BL], f32, tag="htps")
                    for k in range(KH):
                        nc.tensor.transpose(ht_ps[:, k * BL:(k + 1) * BL],
                                            h_sb[:, k * 128:(k + 1) * 128],
                                            id8_sb[:])
                    nc.vector.tensor_copy(
                        hist[d][:, :, t_eff * BL:(t_eff + 1) * BL],
                        ht_ps[:].rearrange("p (k b) -> p k b", k=KH))
                    c_prev = c_new[:]

        # ---------- phase 3: out = sum_d hist[d]^T @ wlin[d] ----------
        with ExitStack() as ph3:
            fp = ph3.enter_context(
                tc.tile_pool(name="fps", bufs=2, space="PSUM"))
            fsb = ph3.enter_context(tc.tile_pool(name="fsb", bufs=2))
            CW = 512 if (s_len * BL) % 512 == 0 else s_len * BL
            NCH = (s_len * BL) // CW
            for j in range(NCH):
                f_ps = fp.tile([T, CW], f32, tag="f")
                for d in range(2):
                    for k in range(KH):
                        nc.tensor.matmul(
                            f_ps[:],
                            wlin_all[:, d * KH + k, :],
                            hist[d][:, k, j * CW:(j + 1) * CW],
                            start=(d == 0 and k == 0),
                            stop=(d == 1 and k == KH - 1))
                f_sb = fsb.tile([T, CW], bf16, tag="fsb")
                nc.vector.tensor_copy(f_sb[:], f_ps[:])
                nc.sync.dma_start(out[:, j * CW:(j + 1) * CW], f_sb[:])

    nc.compile()
    # the PJRT lowering re-serializes the (immutable) module on every call
    # (~0.4s for this program); memoize it.
    _raw = [None]
    _orig = nc.to_json_bytes

    def _cached_json():
        if _raw[0] is None:
            _raw[0] = _orig()
        return _raw[0]

    nc.to_json_bytes = _cached_json
    return nc


def _get_program(cut, steps, s_len=S):
    key = (cut, steps, s_len)
    if key not in _cache:
        _cache[key] = _build_program(cut, steps, s_len)
    return _cache[key]


def _align_idx(start_ids, masks, cut):
    """Gather indices + keep mask for the first `cut` word slots."""
    sid = np.asarray(start_ids)
    msk = np.asarray(masks)
    Bb, Ss = sid.shape
    t = np.arange(cut)[None, :]
    n = (sid >= 0).sum(-1)
    last_sid = np.take_along_axis(sid, (n - 1)[:, None], axis=1)
    sid_c = sid[:, :cut]
    idx = np.where(t == 0, 0,
          np.where(t < n[:, None], sid_c - 1,
          np.where(t == n[:, None], last_sid, 0)))
    idx = np.clip(idx, 0, Ss - 1).astype(np.int64)
    sent_len = msk.sum(-1)
    keep = (t < sent_len[:, None])
    return idx, keep, int(sent_len.max())


def _host_prep(hidden_states, h0, c0, W_ih_f, W_hh_f, b_ih_f, b_hh_f,
               W_ih_b, W_hh_b, b_ih_b, b_hh_b, W_lin, b_lin,
               start_ids, masks, cut):
    import ml_dtypes
    bf16 = ml_dtypes.bfloat16

    hs = np.asarray(hidden_states, np.float32)
    idx, keep, _ = _align_idx(start_ids, masks, cut)
    gathered = np.take_along_axis(hs, idx[:, :, None], axis=1)  # [B,cut,D]
    gathered *= keep[:, :, None]

    # replicated weights -> per-core 1/8 shards (views; concat copies later)
    def bfT(w, perm=None):
        w = np.asarray(w, np.float32).T
        if perm is not None:
            w = w[:, perm]
        return np.ascontiguousarray(w).astype(bf16)

    W_lin = np.asarray(W_lin, np.float32)
    wih_host = np.concatenate([bfT(W_ih_f, _PERM), bfT(W_ih_b, _PERM)], 0)
    whh_host = np.concatenate([bfT(W_hh_f, _PERM), bfT(W_hh_b, _PERM)], 0)
    wlin_host = np.concatenate(
        [np.ascontiguousarray(W_lin[:, :H].T).astype(bf16),
         np.ascontiguousarray(W_lin[:, H:].T).astype(bf16)], 0)
    bias_host = np.stack(
        [(np.asarray(b_ih_f, np.float32) + np.asarray(b_hh_f, np.float32))[_PERM],
         (np.asarray(b_ih_b, np.float32) + np.asarray(b_hh_b, np.float32))[_PERM]],
        0).astype(bf16)

    wih_shards = wih_host.reshape(NC, 2 * D // NC, G4)
    whh_shards = whh_host.reshape(NC, 2 * H // NC, G4)
    wlin_shards = wlin_host.reshape(NC, 2 * H // NC, T)
    bias_shards = bias_host.reshape(NC, 2 * G4 // NC)

    h0 = np.asarray(h0, np.float32)
    c0a = np.asarray(c0, np.float32)
    id128 = np.eye(128, dtype=bf16)
    id8 = np.eye(8, dtype=np.float32)

    ne = cut * BL * D
    nh = 2 * H * BL
    tail = np.concatenate([id128.ravel()] + [np.zeros(0, bf16)])
    in_maps = []
    for core in range(NC):
        bs = slice(core * BL, (core + 1) * BL)
        nbf = (ne + nh + id128.size + wih_shards[core].size
               + whh_shards[core].size + wlin_shards[core].size
               + bias_shards[core].size)
        buf = np.empty(nbf, bf16)
        # cast + transpose + pack the emb segment in one pass
        buf[:ne].reshape(cut, BL, D)[:] = gathered[bs].transpose(1, 0, 2)
        o = ne
        hseg = buf[o:o + nh].reshape(2, H, BL)
        hseg[0] = h0[0, bs].T
        hseg[1] = h0[1, bs].T
        o += nh
        for part in (id128, wih_shards[core], whh_shards[core],
                     wlin_shards[core], bias_shards[core]):
            buf[o:o + part.size] = part.ravel()
            o += part.size
        pf = np.empty(BL * 2 * H + 64, np.float32)
        pfv = pf[:BL * 2 * H].reshape(BL, 2, H)
        pfv[:, 0, :] = c0a[0, bs]
        pfv[:, 1, :] = c0a[1, bs]
        pf[BL * 2 * H:] = id8.ravel()
        in_maps.append({"packed_bf": buf, "packed_f32": pf})
    return in_maps


def kernel(hidden_states, h0, c0, W_ih_f, W_hh_f, b_ih_f, b_hh_f,
           W_ih_b, W_hh_b, b_ih_b, b_hh_b, W_lin, b_lin, start_ids, masks,
           _trace=False):
    _configure_jax_cache()
    from concourse.bass_utils import run_bass_kernel_spmd

    msk = np.asarray(masks)
    max_sent = int(msk.sum(-1).max())
    cut = CUT if max_sent <= CUT - 1 else S

    in_maps = _host_prep(
        hidden_states, h0, c0, W_ih_f, W_hh_f, b_ih_f, b_hh_f,
        W_ih_b, W_hh_b, b_ih_b, b_hh_b, W_lin, b_lin, start_ids, masks, cut)

    nc = _get_program(cut, STEPS)
    res = run_bass_kernel_spmd(nc, in_maps, list(range(NC)), trace=_trace)
    outs = res.results

    b_lin = np.asarray(b_lin, np.float32)
    feats = np.empty((B, S, T), np.float32)
    for core in range(NC):
        o = outs[core]["out"].astype(np.float32)       # [T, S*BL]
        o = o.reshape(T, S, BL).transpose(2, 1, 0)     # [BL, S, T]
        feats[core * BL:(core + 1) * BL] = o + b_lin
    if _trace:
        return feats, res
    return feats
